# revision 1
# baseline (speedup 1.0000x reference)
"""Trainium2 Bass kernel for CausalLocalBlock.

Reference computation (B=4, N=4096, D=256, W=7, K=15, H=1024):
    mix = causal_conv1d(x, w_mix, left_pad=2W) + b_mix
    h   = layer_norm(x + mix) * g1 + b1
    ff  = gelu(h @ w_ff1 + b_ff1) @ w_ff2 + b_ff2
    out = layer_norm(h + ff) * g2 + b2

Sharding: 8 cores, core c handles batch c//2, sequence half c%2 (2048
tokens) with a 14-token halo passed in from the host (no collectives).

On-chip layout is D-major (features on partitions, tokens on the free
dim); the host pre-transposes shards and packs weights so every DMA is
contiguous per partition.  LayerNorm statistics are computed with
ones-matmuls on the PE (partition-dim reduction + broadcast in one op);
rstd is Exp(-0.5*Ln(SQ - S^2/D + D*eps) + 0.5*ln(D)) so both LN phases
stay in the natural_log_exp ACT table set.  Residuals and biases are
folded into extra PE accumulation taps:
  - w_mix[14] += I                      (x + mix residual)
  - lhsT=diag(g1), rhs=hnorm            (h = hnorm*g1 + b1 residual into ff2)
  - lhsT=(b1+b_ff2) row, rhs=ones row   (bias tap)
  - g1 folded into w_ff1, c1 = b1@w_ff1 + b_ff1 folded into gelu bias

Matmuls run in float32r (1 PE cycle/row vs 4 for float32; ~11-bit
mantissa, fp32 PSUM accumulation).  Inputs are pre-rounded on the host
(RNE to the fp32r grid, bit-identical to the HW rounding) so DMAs land
directly in fp32r tiles; on-chip producers of matmul operands write
fp32r so walrus' rounding check passes.

The per-chunk work is emitted software-pipelined so each cross-engine
LayerNorm chain is covered by another chunk's matmul block, and input
DMAs are split (weights by tap group on the sync queue, x by chunk on
the scalar queue) so the conv starts as soon as the first pieces land.

This walrus build encodes at most ONE sync-wait command per
instruction, so `split_multiwaits` hoists extra waits onto single-wait
NoOps after Tile scheduling.
"""

import copy
import math
import sys

if "/opt/trn_rl_repo" not in sys.path:
    sys.path.insert(0, "/opt/trn_rl_repo")

import numpy as np

import concourse.bass as bass
import concourse.mybir as mybir
import concourse.tile as tile
from concourse.bass_utils import run_bass_kernel_spmd

B, N, D, W = 4, 4096, 256, 7
K = 2 * W + 1
H = 4 * D
EPS = 1e-5
NCORES = 8
TOK = B * N // NCORES          # 2048 tokens per core
HALO = 2 * W                   # 14
CHUNK = 512
NCHUNK = TOK // CHUNK          # 4
DH = D // 128                  # 2 partition halves of D
HJ = H // 128                  # 8 partition tiles of H
XC = CHUNK + HALO              # per-chunk x slice width

F32 = mybir.dt.float32
F32R = mybir.dt.float32r
ACTF = mybir.ActivationFunctionType
OP = mybir.AluOpType


def round_fp32r(a):
    """Host-side RNE to the fp32r grid (low 12 mantissa bits dropped)."""
    u = np.ascontiguousarray(a, np.float32).view(np.uint32)
    r = (u.astype(np.uint64) + 0x7FF + ((u >> 12) & 1)) & 0xFFFFF000
    return r.astype(np.uint32).view(np.float32)


def split_multiwaits(nc, max_waits=1):
    """This container's walrus encodes at most one sync-wait command per
    instruction; hoist extra waits onto preceding single-wait NoOps."""
    n = 0
    new_module = copy.replace(nc.m, functions=[])
    for function in nc.m.functions:
        new_function = copy.replace(function, blocks=[])
        new_function.set_allocations_from_list(function.allocations)
        for block in function.blocks:
            new_insts = []
            for inst in block.instructions:
                si = inst.sync_info
                if si is not None and len(si.on_wait) > max_waits:
                    waits = list(si.on_wait)
                    for w in waits[:-max_waits]:
                        n += 1
                        nop = mybir.InstNoOp(name=f"WSPLIT-{n}", ins=[], outs=[])
                        nop.engine = inst.engine
                        nop.sync_info = mybir.SyncInfo(on_wait=[w], on_update=[])
                        new_insts.append(nop)
                    inst.sync_info = mybir.SyncInfo(
                        on_wait=waits[-max_waits:], on_update=list(si.on_update)
                    )
                new_insts.append(inst)
            new_function.blocks.append(copy.replace(block, instructions=new_insts))
        new_module.functions.append(new_function)
    nc.m = new_module
    return n


def build_nc():
    nc = bass.Bass()

    xT = nc.declare_dram_parameter("xT", [D, HALO + TOK], F32, isOutput=False)
    wmix = nc.declare_dram_parameter("wmix", [128, K * DH * DH * 128], F32, isOutput=False)
    w1 = nc.declare_dram_parameter("w1", [128, DH * HJ * 128], F32, isOutput=False)
    w2 = nc.declare_dram_parameter("w2", [128, HJ * DH * 128], F32, isOutput=False)
    dg1 = nc.declare_dram_parameter("dg1", [128, DH * 128], F32, isOutput=False)
    # vecs columns: bmix(2), c1(8), g2(2), b2(2)
    vecs = nc.declare_dram_parameter("vecs", [128, 16], F32, isOutput=False)
    outT = nc.declare_dram_parameter("outT", [D, TOK], F32, isOutput=True)

    xT_v = xT.rearrange("(h p) t -> p h t", p=128).bitcast(F32R)
    wmix_v = wmix.rearrange("p (k a b j) -> p k a b j", k=K, a=DH, b=DH).bitcast(F32R)
    outT_v = outT.rearrange("(h p) t -> p h t", p=128)

    inv_d = 1.0 / D
    exp_bias = 0.5 * math.log(D)   # rstd = exp(-0.5*ln(T) + 0.5*ln(D))

    with tile.TileContext(nc) as tc:
        with tc.tile_pool(name="persist", bufs=1) as pers:
            # --- weights / constants, split DMAs across two HWDGE queues ---
            vecs_sb = pers.tile([128, 16], F32)
            nc.scalar.dma_start(out=vecs_sb, in_=vecs[:, :])
            x_sb = []
            for c in range(NCHUNK):
                xt = pers.tile([128, DH, XC], F32R, tag=f"x{c}")
                nc.scalar.dma_start(
                    out=xt, in_=xT_v[:, :, c * CHUNK : c * CHUNK + XC]
                )
                x_sb.append(xt)

            wmix_sb = pers.tile([128, K, DH, DH, 128], F32R)
            k_edges = [0, 2, 4, 6, 8, 10, 12, 14, K]
            for k0, k1 in zip(k_edges[:-1], k_edges[1:]):
                nc.sync.dma_start(out=wmix_sb[:, k0:k1], in_=wmix_v[:, k0:k1])

            def wmix_tap(ki, di, do):
                return wmix_sb[:, ki, di, do, :]
            w1_sb = pers.tile([128, DH, HJ, 128], F32R)
            nc.sync.dma_start(
                out=w1_sb, in_=w1.rearrange("p (a j n) -> p a j n", a=DH, j=HJ).bitcast(F32R)
            )
            w2_sb = pers.tile([128, HJ, DH, 128], F32R)
            nc.sync.dma_start(
                out=w2_sb, in_=w2.rearrange("p (j a n) -> p j a n", j=HJ, a=DH).bitcast(F32R)
            )
            dg1_sb = pers.tile([128, DH, 128], F32R)
            nc.sync.dma_start(
                out=dg1_sb, in_=dg1.rearrange("p (a n) -> p a n", a=DH).bitcast(F32R)
            )

            bmix_c = vecs_sb[:, 0:2]
            c1_c = vecs_sb[:, 2:10]
            g2_c = vecs_sb[:, 10:12]
            b2_c = vecs_sb[:, 12:14]
            brow_c = vecs_sb[:, 14:16]

            ones_f32 = pers.tile([128, 128], F32)
            nc.vector.memset(ones_f32, 1.0)
            ones_sb = pers.tile([128, 128], F32R)
            nc.vector.tensor_copy(ones_sb, ones_f32)
            deps_col = pers.tile([128, 1], F32)
            nc.vector.memset(deps_col, float(D) * EPS)
            ebias_col = pers.tile([128, 1], F32)
            nc.vector.memset(ebias_col, 0.5 * math.log(D))

            # hnorm (LN1 normalized, pre-g1/b1) and o = h + ff, whole shard
            h_sb = pers.tile([128, DH, TOK], F32R)
            o_sb = pers.tile([128, DH, TOK], F32R)

            with (
                tc.tile_pool(name="big_ps", bufs=2, space="PSUM") as big_ps,
                tc.tile_pool(name="small_ps", bufs=4, space="PSUM") as small_ps,
                tc.tile_pool(name="work", bufs=2) as work,
            ):
                # per-chunk state handed between stage emitters
                st = [dict() for _ in range(NCHUNK)]

                def conv_block(c):
                    yps = big_ps.tile([128, DH, CHUNK], F32, tag="big")
                    st[c]["yps"] = yps
                    for do in range(DH):
                        i, n_mm = 0, K * DH
                        for ki in range(K):
                            for di in range(DH):
                                nc.tensor.matmul(
                                    yps[:, do, :],
                                    wmix_tap(ki, di, do),
                                    x_sb[c][:, di, ki : ki + CHUNK],
                                    start=(i == 0),
                                    stop=(i == n_mm - 1),
                                )
                                i += 1

                def ln_stats(c, src, src_psum, pfx):
                    """Emit sq, stat matmuls, and the T = SQ - S^2/D row for
                    `src` ([128, DH, CHUNK] fp32r SBUF view).  If src_psum is
                    given, also copy it into src (+bmix) on ScalarE first."""
                    if src_psum is not None:
                        for a in range(DH):
                            nc.scalar.activation(
                                src[:, a, :], src_psum[:, a, :], ACTF.Identity,
                                bias=bmix_c[:, a : a + 1], scale=1.0,
                            )
                    sq = work.tile([128, DH, CHUNK], F32R, tag="sq")
                    for a in range(DH):
                        if src_psum is not None:
                            nc.scalar.square(sq[:, a, :], src[:, a, :].bitcast(F32))
                        else:
                            nc.vector.tensor_mul(
                                sq[:, a, :],
                                src[:, a, :].bitcast(F32),
                                src[:, a, :].bitcast(F32),
                            )
                    s_ps = small_ps.tile([128, CHUNK], F32, tag="small")
                    q_ps = small_ps.tile([128, CHUNK], F32, tag="small")
                    for a in range(DH):
                        nc.tensor.matmul(
                            s_ps, ones_sb, src[:, a, :],
                            start=(a == 0), stop=(a == DH - 1),
                        )
                    for a in range(DH):
                        nc.tensor.matmul(
                            q_ps, ones_sb, sq[:, a, :],
                            start=(a == 0), stop=(a == DH - 1),
                        )
                    mu = work.tile([128, CHUNK], F32, tag="mu")
                    nc.vector.tensor_scalar_mul(mu, s_ps, inv_d)
                    t1 = work.tile([128, CHUNK], F32, tag="t1")
                    nc.vector.tensor_mul(t1, mu, s_ps)
                    tv = work.tile([128, CHUNK], F32, tag="tv")
                    nc.vector.tensor_sub(tv, q_ps, t1)
                    st[c][pfx + "mu"] = mu
                    st[c][pfx + "tv"] = tv

                def ln_rstd(c, pfx):
                    # rstd = exp(-0.5*ln(T + D*eps) + 0.5*ln(D))
                    tv = st[c][pfx + "tv"]
                    lnv = work.tile([128, CHUNK], F32, tag="lnv")
                    nc.scalar.activation(lnv, tv, ACTF.Ln, bias=deps_col, scale=1.0)
                    r = work.tile([128, CHUNK], F32, tag="r")
                    nc.scalar.activation(r, lnv, ACTF.Exp, bias=ebias_col, scale=-0.5)
                    st[c][pfx + "r"] = r

                def ln1_apply(c):
                    c0 = c * CHUNK
                    mu, r = st[c]["1mu"], st[c]["1r"]
                    ysb = st[c]["ysb"]
                    mr = work.tile([128, CHUNK], F32, tag="mr")
                    nc.vector.tensor_mul(mr, mu, r)
                    for a in range(DH):
                        t0 = work.tile([128, CHUNK], F32, tag="t0")
                        nc.vector.tensor_mul(t0, ysb[:, a, :].bitcast(F32), r)
                        nc.vector.tensor_sub(h_sb[:, a, c0 : c0 + CHUNK], t0, mr)

                def zg_block(c):
                    c0 = c * CHUNK
                    gel = work.tile([128, HJ, CHUNK], F32R, tag="gel")
                    st[c]["gel"] = gel
                    for j in range(HJ):
                        zps = small_ps.tile([128, CHUNK], F32, tag="small")
                        for di in range(DH):
                            nc.tensor.matmul(
                                zps,
                                w1_sb[:, di, j, :],
                                h_sb[:, di, c0 : c0 + CHUNK],
                                start=(di == 0), stop=(di == DH - 1),
                            )
                        nc.scalar.activation(
                            gel[:, j, :], zps, ACTF.Gelu,
                            bias=c1_c[:, j : j + 1], scale=1.0,
                        )

                def ff2_block(c):
                    c0 = c * CHUNK
                    gel = st[c]["gel"]
                    ops = big_ps.tile([128, DH, CHUNK], F32, tag="big")
                    for do in range(DH):
                        nc.tensor.matmul(
                            ops[:, do, :], dg1_sb[:, do, :],
                            h_sb[:, do, c0 : c0 + CHUNK],
                            start=True, stop=False,
                        )
                        for j in range(HJ):
                            nc.tensor.matmul(
                                ops[:, do, :], w2_sb[:, j, do, :], gel[:, j, :],
                                start=False, stop=(j == HJ - 1),
                            )
                    for a in range(DH):
                        nc.scalar.activation(
                            o_sb[:, a, c0 : c0 + CHUNK], ops[:, a, :], ACTF.Identity,
                            bias=brow_c[:, a : a + 1], scale=1.0,
                        )

                def ln2_apply(c):
                    c0 = c * CHUNK
                    mu, r = st[c]["2mu"], st[c]["2r"]
                    mr = work.tile([128, CHUNK], F32, tag="mr")
                    nc.vector.tensor_mul(mr, mu, r)
                    out_t = work.tile([128, DH, CHUNK], F32, tag="outsb")
                    for a in range(DH):
                        t0 = work.tile([128, CHUNK], F32, tag="t0")
                        nc.vector.tensor_mul(
                            t0, o_sb[:, a, c0 : c0 + CHUNK].bitcast(F32), r
                        )
                        nc.vector.tensor_sub(t0, t0, mr)
                        nc.vector.tensor_scalar(
                            out=out_t[:, a, :], in0=t0,
                            scalar1=g2_c[:, a : a + 1], scalar2=b2_c[:, a : a + 1],
                            op0=OP.mult, op1=OP.add,
                        )
                    nc.sync.dma_start(out=outT_v[:, :, c0 : c0 + CHUNK], in_=out_t)

                def s1(c):
                    ysb = work.tile([128, DH, CHUNK], F32R, tag="ysb")
                    st[c]["ysb"] = ysb
                    ln_stats(c, ysb, st[c]["yps"], "1")

                def s2(c):
                    c0 = c * CHUNK
                    ln_stats(c, o_sb[:, :, c0 : c0 + CHUNK], None, "2")

                # --- software-pipelined emission ---
                conv_block(0)
                conv_block(1)
                s1(0); ln_rstd(0, "1"); ln1_apply(0)
                s1(1); ln_rstd(1, "1"); ln1_apply(1)
                conv_block(2)
                zg_block(0)
                conv_block(3)
                s1(2); ln_rstd(2, "1"); ln1_apply(2)
                s1(3); ln_rstd(3, "1"); ln1_apply(3)
                ff2_block(0)
                zg_block(1)
                ff2_block(1)
                s2(0); ln_rstd(0, "2"); ln2_apply(0)
                zg_block(2)
                ff2_block(2)
                s2(1); ln_rstd(1, "2"); ln2_apply(1)
                s2(2)
                zg_block(3)
                ff2_block(3)
                ln_rstd(2, "2"); ln2_apply(2)
                s2(3); ln_rstd(3, "2"); ln2_apply(3)

    split_multiwaits(nc)
    return nc


def _pack_inputs(x, w_mix, b_mix, g1, b1, w_ff1, b_ff1, w_ff2, b_ff2, g2, b2):
    """Host-side packing shared by all cores (weights) + per-core shards."""
    f32 = np.float32
    f64 = np.float64
    Wm = np.array(w_mix, dtype=f64).copy()
    Wm[K - 1] += np.eye(D)
    wmix_p = round_fp32r(
        Wm.reshape(K, DH, 128, DH, 128).transpose(2, 0, 1, 3, 4).reshape(128, -1)
    )
    W1g = np.array(g1, f64)[:, None] * np.array(w_ff1, f64)
    w1_p = round_fp32r(
        W1g.reshape(DH, 128, HJ, 128).transpose(1, 0, 2, 3).reshape(128, -1)
    )
    w2_p = round_fp32r(
        np.array(w_ff2, f64).reshape(HJ, 128, DH, 128).transpose(1, 0, 2, 3).reshape(128, -1)
    )
    dg1_p = np.zeros((128, DH, 128), f32)
    for a in range(DH):
        dg1_p[np.arange(128), a, np.arange(128)] = np.array(g1, f32)[a * 128 : (a + 1) * 128]
    dg1_p = round_fp32r(dg1_p.reshape(128, -1))
    c1 = (np.array(b1, f64) @ np.array(w_ff1, f64) + np.array(b_ff1, f64)).astype(f32)
    vecs_p = np.zeros((128, 16), f32)
    vecs_p[:, 0:2] = np.array(b_mix, f32).reshape(DH, 128).T
    vecs_p[:, 2:10] = c1.reshape(HJ, 128).T
    vecs_p[:, 10:12] = np.array(g2, f32).reshape(DH, 128).T
    vecs_p[:, 12:14] = np.array(b2, f32).reshape(DH, 128).T
    vecs_p[:, 14:16] = (
        (np.array(b1, f64) + np.array(b_ff2, f64)).astype(f32).reshape(DH, 128).T
    )

    shared = {
        "wmix": wmix_p, "w1": w1_p, "w2": w2_p, "dg1": dg1_p,
        "vecs": vecs_p,
    }
    in_maps = []
    x = np.array(x, f32)
    for core in range(NCORES):
        b, half = divmod(core, 2)
        start = half * TOK
        xT_shard = np.zeros((D, HALO + TOK), f32)
        xT_shard[:, HALO:] = x[b, start : start + TOK].T
        if start > 0:
            xT_shard[:, :HALO] = x[b, start - HALO : start].T
        in_maps.append({"xT": round_fp32r(xT_shard), **shared})
    return in_maps


_NC_CACHE = None


def _get_nc():
    global _NC_CACHE
    if _NC_CACHE is None:
        _NC_CACHE = build_nc()
    return _NC_CACHE


def run_spmd(in_maps, **kwargs):
    return run_bass_kernel_spmd(_get_nc(), in_maps, core_ids=list(range(NCORES)), **kwargs)


def assemble(results):
    out = np.empty((B, N, D), np.float32)
    for core in range(NCORES):
        b, half = divmod(core, 2)
        start = half * TOK
        out[b, start : start + TOK, :] = results[core]["outT"].T
    return out


def kernel(**inputs):
    res = run_spmd(_pack_inputs(**inputs))
    return assemble(res.results)

